# revision 10
# baseline (speedup 1.0000x reference)
"""Causal multi-head self-attention (B=4, T=2048, C=1024, H=16) on 8 TRN2 NeuronCores.

Sharding: core = b*2 + g  (b = batch 0..3, g = head-group 0..1 of 8 heads each).
Data parallel over batch; tensor parallel over heads (column-parallel W_attn,
row-parallel W_proj). Each core returns a partial (T, C) output; the host sums
the two partials per batch (the TP all-reduce happens in the unshard step).

Per-core device kernel (bf16 matmuls, f32 accumulation), per 512-wide q chunk:
  1. qT/kT projection with heads on partitions; head pairs share a 128-row tile
  2. v projection in natural [t, c] layout, strided-copied into a 65-stride
     augmented layout with an all-ones column per head (the ones column turns
     the softmax denominator into row 64 of the y^T psum)
  3. attention in transposed [s, q] layout -- no transposes anywhere:
       S^T block = (kT block)^T @ qT chunk  (head-even rows 0:64 / head-odd rows
       64:128 of the PE array run concurrently: disjoint row groups)
       E = exp(S/8) on ScalarE (no max-subtraction: |scores|/8 < ~7)
       causal mask = precomputed multiplicative 0/1 tile on diagonal blocks
       y^T_aug accumulates v_aug^T @ E over s blocks in PSUM.  Head A's AV is
       row-split into two concurrent K=64 matmuls (same 64x128 tiling mode as
       the S matmuls) accumulating into two banks summed at pair_end; head B
       keeps the full-K form (PSUM budget: 1 mm + 4 s + 3 av = 8 banks).
  4. row-parallel output projection of the finished q chunk.

v3 scheduling: a PE warmup burst of dummy matmuls runs during the initial DMA
wait so the HAM clock gate reaches 8/8 before the first real matmul.  The boot
projection covers only {m0,m4,m1,m5,m2} k-outer plus v block 0; everything
else drains as fillers inside the ACT-paced attention stretches at a higher
feed rate than v2, with outproj due-tags retuned so the qc3 tail stays dense.
"""

import numpy as np
import ml_dtypes

B, T, C, H = 4, 2048, 1024, 16
HS = C // H          # 64
NHL = 8              # local heads per core
KT = C // 128        # 8 contraction subtiles
NQC = T // 512       # 4 query chunks
NTB = T // 128       # 16 t-blocks
Bb16 = ml_dtypes.bfloat16

_CACHE = {}


def _build():
    import concourse.bass as bass
    import concourse.bacc as bacc
    import concourse.tile as tile
    import concourse.mybir as mybir
    from collections import deque

    BF = mybir.dt.bfloat16
    F32 = mybir.dt.float32
    AF = mybir.ActivationFunctionType

    nc = bacc.Bacc("TRN2", target_bir_lowering=False, debug=False, num_devices=8)
    xT = nc.dram_tensor("xT", [C, T], BF, kind="ExternalInput").ap()
    wqk = nc.dram_tensor("wqk", [C, 1024], BF, kind="ExternalInput").ap()
    wv = nc.dram_tensor("wv", [C, 512], BF, kind="ExternalInput").ap()
    wp = nc.dram_tensor("wp", [512, C], BF, kind="ExternalInput").ap()
    mask = nc.dram_tensor("mask", [128, 1280], BF, kind="ExternalInput").ap()
    out = nc.dram_tensor("out", [T, C], BF, kind="ExternalOutput").ap()

    MOFF = [0, 512, 896, 1152]   # mask_sb offsets for diag blocks d=0..3

    with tile.TileContext(nc) as tc:
        with tc.tile_pool(name="persist", bufs=1) as persist, \
             tc.tile_pool(name="mm", bufs=1, space="PSUM") as mmpool, \
             tc.tile_pool(name="s", bufs=2, space="PSUM") as spool, \
             tc.tile_pool(name="av", bufs=3, space="PSUM") as avpool, \
             tc.tile_pool(name="e", bufs=6) as epool, \
             tc.tile_pool(name="nrm", bufs=3) as nrmpool, \
             tc.tile_pool(name="osb", bufs=3) as outpool:

            xT_sb = persist.tile([128, KT, T], BF, tag="xT")
            wqk_sb = persist.tile([128, KT, 1024], BF, tag="wqk")
            wv_sb = persist.tile([128, KT, 512], BF, tag="wv")
            wp_sb = persist.tile([128, 4, 1024], BF, tag="wp")
            mask_sb = persist.tile([128, 1280], BF, tag="mask")
            qk_sb = persist.tile([128, 8, T], BF, tag="qk")
            v_sb = persist.tile([128, NTB, 520], BF, tag="v")
            yT_sb = persist.tile([128, 4, T], BF, tag="yT")
            scr_sb = persist.tile([128, 64], BF, tag="scratch")
            # pair-broadcast weights: w[0,0:64]=w[64,0:64]=1 selects rA
            # into psum partitions 0:64, w[32,64:128]=w[96,64:128]=1 selects rB
            # into 64:128; K=33 slices at base 0 / base 64 match r4's rows
            ones33_sb = persist.tile([97, 128], F32, tag="ones33")
            nc.vector.memset(scr_sb[:], 0.0)
            nc.vector.memset(ones33_sb[:], 0.0)
            for r in (0, 64):
                nc.vector.memset(ones33_sb[r:r + 1, 0:64], 1.0)
                nc.vector.memset(ones33_sb[r + 32:r + 33, 64:128], 1.0)

            # HAM warmup: ~72 dummy matmuls (~4.3us cold) with no DMA deps so
            # the PE clock gate flips to 8/8 while the first inputs stream in
            wm_ps = mmpool.tile([64, 64], F32, tag="mm", name="warmup")
            for _ in range(72):
                nc.tensor.matmul(wm_ps[:], scr_sb[:], scr_sb[:],
                                 start=True, stop=True)

            # load order: exactly what the first projection chunk needs, first
            for k in range(KT):
                nc.sync.dma_start(wqk_sb[:, k, :], wqk[k * 128:(k + 1) * 128, :])
                nc.sync.dma_start(xT_sb[:, k, 0:512], xT[k * 128:(k + 1) * 128, 0:512])
            for k in range(KT):
                nc.sync.dma_start(wv_sb[:, k, :], wv[k * 128:(k + 1) * 128, :])
            nc.sync.dma_start(mask_sb[:], mask[:])
            for k in range(KT):
                nc.sync.dma_start(xT_sb[:, k, 512:1024],
                                  xT[k * 128:(k + 1) * 128, 512:1024])
            for k in range(4):
                nc.sync.dma_start(wp_sb[:, k, :], wp[k * 128:(k + 1) * 128, :])
            for k in range(KT):
                nc.sync.dma_start(xT_sb[:, k, 1024:2048],
                                  xT[k * 128:(k + 1) * 128, 1024:2048])

            # ---- filler generators: ~one PE matmul per step ----
            def qk_group_gen(qc, m):
                q0 = qc * 512
                mm_ps = mmpool.tile([128, 512], F32, tag="mm",
                                    name=f"qkg_{qc}_{m}")
                for k in range(KT):
                    nc.tensor.matmul(
                        mm_ps[:], wqk_sb[:, k, m * 128:(m + 1) * 128],
                        xT_sb[:, k, q0:q0 + 512],
                        start=(k == 0), stop=(k == KT - 1))
                    yield
                nc.vector.tensor_copy(qk_sb[:, m, q0:q0 + 512], mm_ps[:])
                yield

            def v_group_gen(j):
                jj = j * 128
                vps = mmpool.tile([128, 512], F32, tag="mm", name=f"vg_{j}")
                for k in range(KT):
                    nc.tensor.matmul(
                        vps[:], xT_sb[:, k, jj:jj + 128],
                        wv_sb[:, k, :],
                        start=(k == 0), stop=(k == KT - 1))
                    yield
                v3 = v_sb[:, j, :].rearrange("p (h e) -> p h e", e=65)
                with nc.allow_low_precision(reason="v bf16"):
                    nc.vector.tensor_copy(
                        v3[:, :, 0:64],
                        vps[:].rearrange("p (h e) -> p h e", e=64))
                yield
                nc.vector.memset(v3[:, :, 64], 1.0)
                yield

            def outproj_group_gen(qc, tt):
                t0 = (qc * 4 + tt) * 128
                osb = outpool.tile([128, 1024], BF, tag="osb", name=f"og_{qc}_{tt}")
                for n in range(2):
                    ops = mmpool.tile([128, 512], F32, tag="mm",
                                      name=f"op_{qc}_{tt}_{n}")
                    for cp in range(4):
                        nc.tensor.matmul(
                            ops[:], yT_sb[:, cp, t0:t0 + 128],
                            wp_sb[:, cp, n * 512:(n + 1) * 512],
                            start=(cp == 0), stop=(cp == 3))
                        yield
                    with nc.allow_low_precision(reason="bf16 partial output"):
                        nc.vector.tensor_copy(osb[:, n * 512:(n + 1) * 512], ops[:])
                    yield
                nc.sync.dma_start(out[t0:t0 + 128, :], osb[:])

            # ---- producer bookkeeping ----
            # gens: one generator per projection group, drained cooperatively
            # by feed() or force-finished by require() right before the first
            # consumer is emitted (the tile framework orders dependencies by
            # emission order, so producers MUST be emitted first).
            gens = {}
            done = set()
            hard_q = deque()         # (due, key): qkv projection groups
            soft_q = deque()         # (due, key): outproj groups
            active = []              # [key] mid-group: finish it before
                                     # starting another (mm pool is 1 buf;
                                     # interleaving two psum-holding groups
                                     # stalls the PE)

            def _step(key):
                try:
                    next(gens[key])
                    return True
                except StopIteration:
                    done.add(key)
                    if active and active[0] == key:
                        active.clear()
                    return False

            def require(key):
                if key in done:
                    return
                if active and active[0] != key:
                    while _step(active[0]):
                        pass
                if not active:
                    active.append(key)
                while _step(key):
                    pass

            def feed(now, n):
                while n > 0:
                    if active:
                        key = active[0]
                    else:
                        while soft_q and soft_q[0][1] in done:
                            soft_q.popleft()
                        while hard_q and hard_q[0][1] in done:
                            hard_q.popleft()
                        if soft_q and soft_q[0][0] <= now:
                            key = soft_q.popleft()[1]
                            active.append(key)
                        elif hard_q and hard_q[0][0] <= now + 1:
                            # hard proj groups may run one chunk early
                            key = hard_q.popleft()[1]
                            active.append(key)
                        else:
                            return
                    if _step(key):
                        n -= 1

            def add_hard(due, key, gen):
                gens[key] = gen
                hard_q.append((due, key))

            def add_proj_chunk(qc):
                # consumer order: q slots m0/m4 and the new v blocks feed
                # hp0 first; later head pairs follow
                add_hard(qc, ("qk", qc, 0), qk_group_gen(qc, 0))
                add_hard(qc, ("qk", qc, 4), qk_group_gen(qc, 4))
                for j in range(4 * qc, 4 * qc + 4):
                    add_hard(qc, ("v", j), v_group_gen(j))
                for m in (1, 5, 2, 6, 3, 7):
                    add_hard(qc, ("qk", qc, m), qk_group_gen(qc, m))

            def drain_soft():
                while active:
                    if not _step(active[0]):
                        break
                while soft_q:
                    due, key = soft_q.popleft()
                    if key in done:
                        continue
                    while _step(key):
                        pass

            # ---- attention units: one kv block, both heads of a pair ----
            def unit_list(qc):
                units = []
                for hp in range(4):
                    blocks = [(j, 0, 512) for j in range(4 * qc)] + \
                             [(4 * qc + d, 128 * d, 512 - 128 * d) for d in range(4)]
                    nb = len(blocks)
                    for bi, (j, qo, w) in enumerate(blocks):
                        units.append((hp, j, qo, w, bi == 0, bi == nb - 1))
                return units

            def emit_S(qc, u):
                hp, j, qo, w, first, last = u
                q0 = qc * 512
                s = spool.tile([128, 1024], F32, tag="s",
                               name=f"s_{qc}_{hp}_{j}")
                # the two heads target disjoint PE row groups, so they stream
                # through the array concurrently
                for pb, off in ((0, 0), (64, 512)):
                    nc.tensor.matmul(
                        s[:, off:off + w],
                        qk_sb[pb:pb + 64, 4 + hp, j * 128:(j + 1) * 128],
                        qk_sb[pb:pb + 64, hp, q0 + qo:q0 + 512],
                        start=True, stop=True,
                        tile_position=(pb, 0))
                return s

            def emit_expmask(qc, u, s):
                hp, j, qo, w, first, last = u
                e = epool.tile([128, 1024], BF, tag="e", name=f"e_{qc}_{hp}_{j}")
                s3 = s.rearrange("p (g q) -> p g q", g=2)[:, :, 0:w]
                e3 = e.rearrange("p (g q) -> p g q", g=2)[:, :, 0:w]
                nc.scalar.activation(e3, s3, AF.Exp, scale=0.125)
                if j >= 4 * qc:
                    moff = MOFF[j - 4 * qc]
                    for off in (0, 512):
                        nc.vector.tensor_mul(
                            e[:, off:off + w], e[:, off:off + w],
                            mask_sb[:, moff:moff + w])
                return e

            def emit_AVA(qc, u, e, avAt, avAb):
                # head A (even head of the pair): row-split K=64 pair in the
                # same 64x128 tiling mode as the S matmuls; the two halves
                # stream concurrently into separate banks
                hp, j, qo, w, first, last = u
                h = 2 * hp
                for av, pb in ((avAt, 0), (avAb, 64)):
                    nc.tensor.matmul(
                        av[:, qo:512],
                        v_sb[pb:pb + 64, j, h * 65:h * 65 + 65],
                        e[pb:pb + 64, 0:w],
                        start=first, stop=last,
                        tile_position=(pb, 0))

            def emit_AVB(qc, u, e, avB):
                hp, j, qo, w, first, last = u
                h = 2 * hp + 1
                nc.tensor.matmul(
                    avB[:, qo:512], v_sb[:, j, h * 65:h * 65 + 65],
                    e[:, 512:512 + w],
                    start=first, stop=last)

            def pair_end(qc, hp, avAt, avAb, avB, yraw_sb, den8_sb):
                # stash y and denominator; av psum slots free right away.
                # head A: sum of the two row-split banks (DVE reads at most
                # one PSUM operand, so evacuate the bottom bank first);
                # head B: plain copy
                hA, hB = 2 * hp, 2 * hp + 1
                ab_sb = nrmpool.tile([65, 512], F32, tag="avab",
                                     name=f"avab_{qc}_{hp}")
                nc.vector.tensor_copy(ab_sb[:], avAb[:])
                with nc.allow_low_precision(reason="attention y bf16"):
                    nc.vector.tensor_add(
                        yraw_sb[:, hA, :], avAt[0:64, :], ab_sb[0:64, :])
                    nc.vector.tensor_copy(yraw_sb[:, hB, :], avB[0:64, :])
                pA = (hA % 4) * 32
                pB = (hB % 4) * 32
                nc.vector.tensor_add(
                    den8_sb[pA:pA + 1, hA // 4, :], avAt[64:65, :],
                    ab_sb[64:65, :])
                nc.vector.tensor_copy(
                    den8_sb[pB:pB + 1, hB // 4, :], avB[64:65, :])

            def normalize_half(qc, half, yraw_sb, den8_sb):
                # heads 4*half .. 4*half+3 finished: reciprocal + scale them.
                # One K=33 fp32 matmul per head-pair broadcasts both heads'
                # reciprocal rows straight out of r4 (rows 1-31 are finite 1.0
                # from the den8 memset, zero-weighted).
                q0 = qc * 512
                r4_sb = nrmpool.tile([128, 512], F32, tag="r4",
                                     name=f"r4_{qc}_{half}")
                nc.vector.reciprocal_approx_fast(r4_sb[:], den8_sb[:, half, :])
                for hp in (2 * half, 2 * half + 1):
                    base = (hp % 2) * 64
                    bc_ps = avpool.tile([128, 512], F32, tag="av",
                                        name=f"bc_{qc}_{hp}")
                    with nc.allow_low_precision(reason="fp32r broadcast"):
                        nc.tensor.matmul(
                            bc_ps[:], ones33_sb[base:base + 33, :],
                            r4_sb[base:base + 33, :], start=True, stop=True)
                    with nc.allow_low_precision(reason="attention y bf16"):
                        nc.vector.tensor_mul(
                            yT_sb[0:64, hp, q0:q0 + 512],
                            yraw_sb[:, 2 * hp, :], bc_ps[0:64, :])
                        nc.vector.tensor_mul(
                            yT_sb[64:128, hp, q0:q0 + 512],
                            yraw_sb[:, 2 * hp + 1, :], bc_ps[64:128, :])

            # ---- main schedule ----
            # boot: chunk-0 qT/kT projection, k as the OUTER loop over 5
            # column slots (1 mm buf + 4 s-tile halves) so the first matmuls
            # issue after only the first k-slice of DMA; the remaining slots
            # and v blocks 1-3 drain as fillers inside qc0's attention.
            BOOT = [0, 4, 1, 5, 2]
            ps = []
            for idx, m in enumerate(BOOT):
                if idx == 0:
                    ps.append(mmpool.tile([128, 512], F32, tag="mm",
                                          name=f"boot{m}"))
                else:
                    if idx % 2 == 1:
                        st = spool.tile([128, 1024], F32, tag="s",
                                        name=f"boot{m}")
                    ps.append(st[:, ((idx - 1) % 2) * 512:
                                 ((idx - 1) % 2) * 512 + 512])
            for k in range(KT):
                for idx, m in enumerate(BOOT):
                    nc.tensor.matmul(
                        ps[idx], wqk_sb[:, k, m * 128:(m + 1) * 128],
                        xT_sb[:, k, 0:512],
                        start=(k == 0), stop=(k == KT - 1))
            for idx, m in enumerate(BOOT):
                nc.vector.tensor_copy(qk_sb[:, m, 0:512], ps[idx])
                done.add(("qk", 0, m))
            # remaining qc0 projection work, ordered by first consumer:
            # v0-3 feed hp0's AVs, m6 feeds hp2's S, m3/m7 feed hp3's S
            for j in (0, 1, 2, 3):
                add_hard(0, ("v", j), v_group_gen(j))
            for m in (6, 3, 7):
                add_hard(0, ("qk", 0, m), qk_group_gen(0, m))
            add_proj_chunk(1)

            for qc in range(NQC):
                yraw_sb = nrmpool.tile([64, NHL, 512], BF, tag="yraw",
                                       name=f"yraw{qc}")
                den8_sb = nrmpool.tile([128, 2, 512], F32, tag="den8",
                                       name=f"den8{qc}")
                nc.vector.memset(den8_sb[:], 1.0)
                units = unit_list(qc)
                pend = None      # ((unit, e), (unit, e))
                cur = {}

                def flush_pair(pend):
                    (ua, ea), (ub, eb) = pend
                    require(("v", ua[1]))
                    require(("v", ub[1]))
                    if ua[4]:    # first unit of its head pair
                        cur["avAt"] = avpool.tile([65, 512], F32, tag="av",
                                                  name=f"avAt_{qc}_{ua[0]}")
                        cur["avAb"] = avpool.tile([65, 512], F32, tag="av",
                                                  name=f"avAb_{qc}_{ua[0]}")
                        cur["avB"] = avpool.tile([65, 512], F32, tag="av",
                                                 name=f"avB_{qc}_{ua[0]}")
                    # both AVA pairs (64x128 mode, same as S), then both AVBs
                    emit_AVA(qc, ua, ea, cur["avAt"], cur["avAb"])
                    emit_AVA(qc, ub, eb, cur["avAt"], cur["avAb"])
                    emit_AVB(qc, ua, ea, cur["avB"])
                    emit_AVB(qc, ub, eb, cur["avB"])
                    if ub[5]:    # last unit of its head pair
                        hp = ub[0]
                        pair_end(qc, hp, cur["avAt"], cur["avAb"], cur["avB"],
                                 yraw_sb, den8_sb)
                        if hp == 1:
                            normalize_half(qc, 0, yraw_sb, den8_sb)
                        elif hp == 3:
                            normalize_half(qc, 1, yraw_sb, den8_sb)

                # units in twos: both S pairs back-to-back so the second
                # unit's LDWEIGHTS overlaps the first's member-B stream
                # (disjoint row groups); pairs have an even unit count so
                # groups never straddle an hp boundary
                nfeed = 8 if qc == 0 else 3
                for ua, ub in zip(units[0::2], units[1::2]):
                    for u in (ua, ub):
                        require(("qk", qc, u[0]))
                        require(("qk", u[1] // 4, 4 + u[0]))
                    sa = emit_S(qc, ua)
                    sb = emit_S(qc, ub)
                    ea = emit_expmask(qc, ua, sa)
                    eb = emit_expmask(qc, ub, sb)
                    if pend is not None:
                        flush_pair(pend)
                    feed(qc, nfeed)
                    pend = ((ua, ea), (ub, eb))
                flush_pair(pend)
                # stagger outproj due-tags: most groups feed the next chunks'
                # attention; qc3's groups stay for the final drain
                otags = {0: (1, 1, 2, 2), 1: (2, 2, 3, 3),
                         2: (3, 3, 3, 3), 3: (4, 4, 4, 4)}[qc]
                for tt in range(4):
                    key = ("op", qc, tt)
                    gens[key] = outproj_group_gen(qc, tt)
                    soft_q.append((otags[tt], key))
                if qc + 2 < NQC:
                    add_proj_chunk(qc + 2)
            drain_soft()
    nc.compile()
    return nc


def _get_nc():
    if "nc" not in _CACHE:
        _CACHE["nc"] = _build()
    return _CACHE["nc"]


def _host_prep(x, W_attn, W_proj):
    """Shard + lay out per-core inputs. Returns list of 8 in_maps."""
    x = np.asarray(x, dtype=np.float32)
    W_attn = np.asarray(W_attn, dtype=np.float32)
    W_proj = np.asarray(W_proj, dtype=np.float32)

    # triangular mask prefix: mask[s, i] = 1.0 if s <= i else 0
    s_idx = np.arange(128)[:, None]
    q_idx = np.arange(512)[None, :]
    tri = (s_idx <= q_idx).astype(np.float32)
    mask = np.ascontiguousarray(np.concatenate(
        [tri[:, :512], tri[:, :384], tri[:, :256], tri[:, :128]], axis=1
    )).astype(Bb16)

    xT_b = [np.ascontiguousarray(x[b].T).astype(Bb16) for b in range(B)]
    in_maps = []
    for core in range(8):
        b, g = core // 2, core % 2
        c0 = g * 512
        wqk_g = np.concatenate(
            [W_attn[:, c0:c0 + 512], W_attn[:, C + c0:C + c0 + 512]], axis=1
        ).astype(Bb16)
        wv_g = np.ascontiguousarray(
            W_attn[:, 2 * C + c0:2 * C + c0 + 512]).astype(Bb16)
        wp_g = np.ascontiguousarray(W_proj[c0:c0 + 512, :]).astype(Bb16)
        in_maps.append({
            "xT": xT_b[b],
            "wqk": np.ascontiguousarray(wqk_g),
            "wv": wv_g,
            "wp": wp_g,
            "mask": mask,
        })
    return in_maps


def kernel(x, W_attn, W_proj):
    from concourse import bass_utils

    nc = _get_nc()
    in_maps = _host_prep(x, W_attn, W_proj)
    res = bass_utils.run_bass_kernel_spmd(nc, in_maps, core_ids=list(range(8)))
    outs = [res.results[c]["out"] for c in range(8)]
    full = np.empty((B, T, C), dtype=np.float32)
    for b in range(B):
        full[b] = outs[2 * b].astype(np.float32) + outs[2 * b + 1].astype(np.float32)
    return full


# revision 15
# speedup vs baseline: 1.1662x; 1.1662x over previous
"""Causal multi-head self-attention (B=4, T=2048, C=1024, H=16) on 8 TRN2 NeuronCores.

Sharding: core = b*2 + g  (b = batch 0..3, g = head-group 0..1 of 8 heads each).
Data parallel over batch; tensor parallel over heads (column-parallel W_attn,
row-parallel W_proj). Each core returns a partial (T, C) output; the host sums
the two partials per batch (the TP all-reduce happens in the unshard step).

Per-core device kernel (bf16 matmuls, f32 accumulation), per 512-wide q chunk:
  1. qT/kT projection with heads on partitions; head pairs share a 128-row tile
  2. v projection in natural [t, c] layout, strided-copied into a 65-stride
     augmented layout with an all-ones column per head (the ones column turns
     the softmax denominator into row 64 of the y^T psum)
  3. attention in transposed [s, q] layout -- no transposes anywhere:
       S^T block = (kT block)^T @ qT chunk  (head-even rows 0:64 / head-odd rows
       64:128 of the PE array run concurrently: disjoint row groups)
       E = exp(S/8) on ScalarE (no max-subtraction: |scores|/8 < ~7)
       causal mask = precomputed multiplicative 0/1 tile on diagonal blocks
       y^T_aug accumulates v_aug^T @ E over s blocks in PSUM.  Head A's AV is
       row-split into two concurrent K=64 matmuls (same 64x128 tiling mode as
       the S matmuls) accumulating into two banks summed at pair_end; head B
       keeps the full-K form (PSUM budget: 1 mm + 4 s + 3 av = 8 banks).
  4. row-parallel output projection of the finished q chunk.

v3 scheduling: a PE warmup burst of dummy matmuls runs during the initial DMA
wait so the HAM clock gate reaches 8/8 before the first real matmul.  The boot
projection covers only {m0,m4,m1,m5,m2} k-outer plus v block 0; everything
else drains as fillers inside the ACT-paced attention stretches at a higher
feed rate than v2, with outproj due-tags retuned so the qc3 tail stays dense.
"""

import numpy as np
import ml_dtypes

B, T, C, H = 4, 2048, 1024, 16
HS = C // H          # 64
NHL = 8              # local heads per core
KT = C // 128        # 8 contraction subtiles
NQC = T // 512       # 4 query chunks
NTB = T // 128       # 16 t-blocks
Bb16 = ml_dtypes.bfloat16

_CACHE = {}


def _build():
    import concourse.bass as bass
    import concourse.bacc as bacc
    import concourse.tile as tile
    import concourse.mybir as mybir
    from collections import deque

    BF = mybir.dt.bfloat16
    F32 = mybir.dt.float32
    AF = mybir.ActivationFunctionType

    nc = bacc.Bacc("TRN2", target_bir_lowering=False, debug=False, num_devices=8)
    xT = nc.dram_tensor("xT", [C, T], BF, kind="ExternalInput").ap()
    wqk = nc.dram_tensor("wqk", [C, 1024], BF, kind="ExternalInput").ap()
    wv = nc.dram_tensor("wv", [C, 512], BF, kind="ExternalInput").ap()
    wp = nc.dram_tensor("wp", [512, C], BF, kind="ExternalInput").ap()
    mask = nc.dram_tensor("mask", [128, 1280], BF, kind="ExternalInput").ap()
    out = nc.dram_tensor("out", [T, C], BF, kind="ExternalOutput").ap()

    MOFF = [0, 512, 896, 1152]   # mask_sb offsets for diag blocks d=0..3

    with tile.TileContext(nc) as tc:
        with tc.tile_pool(name="persist", bufs=1) as persist, \
             tc.tile_pool(name="mm", bufs=2, space="PSUM") as mmpool, \
             tc.tile_pool(name="s", bufs=2, space="PSUM") as spool, \
             tc.tile_pool(name="av", bufs=2, space="PSUM") as avpool, \
             tc.tile_pool(name="e", bufs=6) as epool, \
             tc.tile_pool(name="nrm", bufs=3) as nrmpool, \
             tc.tile_pool(name="osb", bufs=3) as outpool:

            xT_sb = persist.tile([128, KT, T], BF, tag="xT")
            wqk_sb = persist.tile([128, KT, 1024], BF, tag="wqk")
            wv_sb = persist.tile([128, KT, 512], BF, tag="wv")
            wp_sb = persist.tile([128, 4, 1024], BF, tag="wp")
            mask_sb = persist.tile([128, 1280], BF, tag="mask")
            qk_sb = persist.tile([128, 8, T], BF, tag="qk")
            v_sb = persist.tile([128, NTB, 520], BF, tag="v")
            yT_sb = persist.tile([128, 4, T], BF, tag="yT")
            scr_sb = persist.tile([128, 64], BF, tag="scratch")
            # pair-broadcast weights: w[0,0:64]=w[64,0:64]=1 selects rA
            # into psum partitions 0:64, w[32,64:128]=w[96,64:128]=1 selects rB
            # into 64:128; K=33 slices at base 0 / base 64 match r4's rows
            ones33_sb = persist.tile([97, 128], F32, tag="ones33")
            nc.vector.memset(scr_sb[:], 0.0)
            nc.vector.memset(ones33_sb[:], 0.0)
            for r in (0, 64):
                nc.vector.memset(ones33_sb[r:r + 1, 0:64], 1.0)
                nc.vector.memset(ones33_sb[r + 32:r + 33, 64:128], 1.0)

            # HAM warmup: ~72 dummy matmuls (~4.3us cold) with no DMA deps so
            # the PE clock gate flips to 8/8 while the first inputs stream in
            wm_ps = mmpool.tile([64, 64], F32, tag="mm", name="warmup")
            for _ in range(72):
                nc.tensor.matmul(wm_ps[:], scr_sb[:], scr_sb[:],
                                 start=True, stop=True)

            # load order: exactly what the first projection chunk needs, first
            for k in range(KT):
                nc.sync.dma_start(wqk_sb[:, k, :], wqk[k * 128:(k + 1) * 128, :])
                nc.sync.dma_start(xT_sb[:, k, 0:512], xT[k * 128:(k + 1) * 128, 0:512])
            for k in range(KT):
                nc.sync.dma_start(wv_sb[:, k, :], wv[k * 128:(k + 1) * 128, :])
            nc.sync.dma_start(mask_sb[:], mask[:])
            for k in range(KT):
                nc.sync.dma_start(xT_sb[:, k, 512:1024],
                                  xT[k * 128:(k + 1) * 128, 512:1024])
            for k in range(4):
                nc.sync.dma_start(wp_sb[:, k, :], wp[k * 128:(k + 1) * 128, :])
            for k in range(KT):
                nc.sync.dma_start(xT_sb[:, k, 1024:2048],
                                  xT[k * 128:(k + 1) * 128, 1024:2048])

            # ---- filler generators: ~one PE matmul per step ----
            def qk_group_gen(qc, m):
                q0 = qc * 512
                mm_ps = mmpool.tile([128, 512], F32, tag="mm",
                                    name=f"qkg_{qc}_{m}")
                for k in range(KT):
                    nc.tensor.matmul(
                        mm_ps[:], wqk_sb[:, k, m * 128:(m + 1) * 128],
                        xT_sb[:, k, q0:q0 + 512],
                        start=(k == 0), stop=(k == KT - 1))
                    yield
                nc.vector.tensor_copy(qk_sb[:, m, q0:q0 + 512], mm_ps[:])
                yield

            def v_group_gen(j):
                jj = j * 128
                vps = mmpool.tile([128, 512], F32, tag="mm", name=f"vg_{j}")
                for k in range(KT):
                    nc.tensor.matmul(
                        vps[:], xT_sb[:, k, jj:jj + 128],
                        wv_sb[:, k, :],
                        start=(k == 0), stop=(k == KT - 1))
                    yield
                v3 = v_sb[:, j, :].rearrange("p (h e) -> p h e", e=65)
                with nc.allow_low_precision(reason="v bf16"):
                    nc.vector.tensor_copy(
                        v3[:, :, 0:64],
                        vps[:].rearrange("p (h e) -> p h e", e=64))
                yield
                nc.vector.memset(v3[:, :, 64], 1.0)
                yield

            def outproj_group_gen(qc, tt):
                t0 = (qc * 4 + tt) * 128
                osb = outpool.tile([128, 1024], BF, tag="osb", name=f"og_{qc}_{tt}")
                for n in range(2):
                    ops = mmpool.tile([128, 512], F32, tag="mm",
                                      name=f"op_{qc}_{tt}_{n}")
                    for cp in range(4):
                        nc.tensor.matmul(
                            ops[:], yT_sb[:, cp, t0:t0 + 128],
                            wp_sb[:, cp, n * 512:(n + 1) * 512],
                            start=(cp == 0), stop=(cp == 3))
                        yield
                    with nc.allow_low_precision(reason="bf16 partial output"):
                        nc.vector.tensor_copy(osb[:, n * 512:(n + 1) * 512], ops[:])
                    yield
                nc.sync.dma_start(out[t0:t0 + 128, :], osb[:])

            # ---- producer bookkeeping ----
            # gens: one generator per projection group, drained cooperatively
            # by feed() or force-finished by require() right before the first
            # consumer is emitted (the tile framework orders dependencies by
            # emission order, so producers MUST be emitted first).
            gens = {}
            done = set()
            hard_q = deque()         # (due, key): qkv projection groups
            soft_q = deque()         # (due, key): outproj groups
            active = []              # [key] mid-group: finish it before
                                     # starting another (mm pool is 1 buf;
                                     # interleaving two psum-holding groups
                                     # stalls the PE)

            def _step(key):
                try:
                    next(gens[key])
                    return True
                except StopIteration:
                    done.add(key)
                    if active and active[0] == key:
                        active.clear()
                    return False

            def require(key):
                if key in done:
                    return
                if active and active[0] != key:
                    while _step(active[0]):
                        pass
                if not active:
                    active.append(key)
                while _step(key):
                    pass

            def feed(now, n):
                while n > 0:
                    if active:
                        key = active[0]
                    else:
                        while soft_q and soft_q[0][1] in done:
                            soft_q.popleft()
                        while hard_q and hard_q[0][1] in done:
                            hard_q.popleft()
                        if soft_q and soft_q[0][0] <= now:
                            key = soft_q.popleft()[1]
                            active.append(key)
                        elif hard_q and hard_q[0][0] <= now + 1:
                            # hard proj groups may run one chunk early
                            key = hard_q.popleft()[1]
                            active.append(key)
                        else:
                            return
                    if _step(key):
                        n -= 1

            def add_hard(due, key, gen):
                gens[key] = gen
                hard_q.append((due, key))

            def add_proj_chunk(qc):
                # consumer order: q slots m0/m4 and the new v blocks feed
                # hp0 first; later head pairs follow
                add_hard(qc, ("qk", qc, 0), qk_group_gen(qc, 0))
                add_hard(qc, ("qk", qc, 4), qk_group_gen(qc, 4))
                for j in range(4 * qc, 4 * qc + 4):
                    add_hard(qc, ("v", j), v_group_gen(j))
                for m in (1, 5, 2, 6, 3, 7):
                    add_hard(qc, ("qk", qc, m), qk_group_gen(qc, m))

            def drain_soft():
                while active:
                    if not _step(active[0]):
                        break
                while soft_q:
                    due, key = soft_q.popleft()
                    if key in done:
                        continue
                    while _step(key):
                        pass

            # ---- attention units: one kv block, both heads of a pair ----
            def unit_list(qc):
                units = []
                for hp in range(4):
                    blocks = [(j, 0, 512) for j in range(4 * qc)] + \
                             [(4 * qc + d, 128 * d, 512 - 128 * d) for d in range(4)]
                    nb = len(blocks)
                    for bi, (j, qo, w) in enumerate(blocks):
                        units.append((hp, j, qo, w, bi == 0, bi == nb - 1))
                return units

            def emit_S(qc, u):
                hp, j, qo, w, first, last = u
                q0 = qc * 512
                s = spool.tile([128, 1024], F32, tag="s",
                               name=f"s_{qc}_{hp}_{j}")
                # the two heads target disjoint PE row groups, so they stream
                # through the array concurrently
                for pb, off in ((0, 0), (64, 512)):
                    nc.tensor.matmul(
                        s[:, off:off + w],
                        qk_sb[pb:pb + 64, 4 + hp, j * 128:(j + 1) * 128],
                        qk_sb[pb:pb + 64, hp, q0 + qo:q0 + 512],
                        start=True, stop=True,
                        tile_position=(pb, 0))
                return s

            def emit_expmask(qc, u, s):
                hp, j, qo, w, first, last = u
                e = epool.tile([128, 1024], BF, tag="e", name=f"e_{qc}_{hp}_{j}")
                s3 = s.rearrange("p (g q) -> p g q", g=2)[:, :, 0:w]
                e3 = e.rearrange("p (g q) -> p g q", g=2)[:, :, 0:w]
                nc.scalar.activation(e3, s3, AF.Exp, scale=0.125)
                if j >= 4 * qc:
                    moff = MOFF[j - 4 * qc]
                    for off in (0, 512):
                        nc.vector.tensor_mul(
                            e[:, off:off + w], e[:, off:off + w],
                            mask_sb[:, moff:moff + w])
                return e

            def emit_AV(qc, u, e, avA, avB):
                hp, j, qo, w, first, last = u
                for av, off, h in ((avA, 0, 2 * hp), (avB, 512, 2 * hp + 1)):
                    nc.tensor.matmul(
                        av[:, qo:512], v_sb[:, j, h * 65:h * 65 + 65],
                        e[:, off:off + w],
                        start=first, stop=last)

            def pair_end(qc, hp, avA, avB, yraw_sb, den8_sb):
                # stash y and denominator; av psum slots free right away
                for h, av_ps in ((2 * hp, avA), (2 * hp + 1, avB)):
                    with nc.allow_low_precision(reason="attention y bf16"):
                        nc.vector.tensor_copy(yraw_sb[:, h, :], av_ps[0:64, :])
                    p32 = (h % 4) * 32
                    nc.vector.tensor_copy(
                        den8_sb[p32:p32 + 1, h // 4, :], av_ps[64:65, :])

            def normalize_half(qc, half, yraw_sb, den8_sb):
                # heads 4*half .. 4*half+3 finished: reciprocal + scale them.
                # One K=33 fp32 matmul per head-pair broadcasts both heads'
                # reciprocal rows straight out of r4 (rows 1-31 are finite 1.0
                # from the den8 memset, zero-weighted).
                q0 = qc * 512
                r4_sb = nrmpool.tile([128, 512], F32, tag="r4",
                                     name=f"r4_{qc}_{half}")
                nc.vector.reciprocal_approx_fast(r4_sb[:], den8_sb[:, half, :])
                for hp in (2 * half, 2 * half + 1):
                    base = (hp % 2) * 64
                    bc_ps = avpool.tile([128, 512], F32, tag="av",
                                        name=f"bc_{qc}_{hp}")
                    with nc.allow_low_precision(reason="fp32r broadcast"):
                        nc.tensor.matmul(
                            bc_ps[:], ones33_sb[base:base + 33, :],
                            r4_sb[base:base + 33, :], start=True, stop=True)
                    with nc.allow_low_precision(reason="attention y bf16"):
                        nc.vector.tensor_mul(
                            yT_sb[0:64, hp, q0:q0 + 512],
                            yraw_sb[:, 2 * hp, :], bc_ps[0:64, :])
                        nc.vector.tensor_mul(
                            yT_sb[64:128, hp, q0:q0 + 512],
                            yraw_sb[:, 2 * hp + 1, :], bc_ps[64:128, :])

            # ---- main schedule ----
            # boot: chunk-0 qT/kT projection, k as the OUTER loop over 6
            # column slots (2 mm bufs + 4 s-tile halves) so the first matmuls
            # issue after only the first k-slice of DMA; the remaining slots
            # and v blocks 0-3 drain as fillers inside qc0's attention.
            BOOT = [0, 4, 1, 5, 2, 6]
            ps = []
            for idx, m in enumerate(BOOT):
                if idx < 2:
                    ps.append(mmpool.tile([128, 512], F32, tag="mm",
                                          name=f"boot{m}"))
                else:
                    if idx % 2 == 0:
                        st = spool.tile([128, 1024], F32, tag="s",
                                        name=f"boot{m}")
                    ps.append(st[:, (idx % 2) * 512:(idx % 2) * 512 + 512])
            for k in range(KT):
                for idx, m in enumerate(BOOT):
                    nc.tensor.matmul(
                        ps[idx], wqk_sb[:, k, m * 128:(m + 1) * 128],
                        xT_sb[:, k, 0:512],
                        start=(k == 0), stop=(k == KT - 1))
            for idx, m in enumerate(BOOT):
                nc.vector.tensor_copy(qk_sb[:, m, 0:512], ps[idx])
                done.add(("qk", 0, m))
            # remaining qc0 projection work, ordered by first consumer:
            # v0-3 feed hp0's AVs, m3/m7 feed hp3's S
            for j in (0, 1, 2, 3):
                add_hard(0, ("v", j), v_group_gen(j))
            for m in (3, 7):
                add_hard(0, ("qk", 0, m), qk_group_gen(0, m))
            add_proj_chunk(1)

            for qc in range(NQC):
                yraw_sb = nrmpool.tile([64, NHL, 512], BF, tag="yraw",
                                       name=f"yraw{qc}")
                den8_sb = nrmpool.tile([128, 2, 512], F32, tag="den8",
                                       name=f"den8{qc}")
                nc.vector.memset(den8_sb[:], 1.0)
                units = unit_list(qc)
                pend = None      # ((unit, e), (unit, e))
                cur = {}

                def flush_pair(pend):
                    (ua, ea), (ub, eb) = pend
                    require(("v", ua[1]))
                    require(("v", ub[1]))
                    if ua[4]:    # first unit of its head pair
                        cur["avA"] = avpool.tile([65, 512], F32, tag="av",
                                                 name=f"avA_{qc}_{ua[0]}")
                        cur["avB"] = avpool.tile([65, 512], F32, tag="av",
                                                 name=f"avB_{qc}_{ua[0]}")
                    emit_AV(qc, ua, ea, cur["avA"], cur["avB"])
                    emit_AV(qc, ub, eb, cur["avA"], cur["avB"])
                    if ub[5]:    # last unit of its head pair
                        hp = ub[0]
                        pair_end(qc, hp, cur["avA"], cur["avB"],
                                 yraw_sb, den8_sb)
                        if hp == 1:
                            normalize_half(qc, 0, yraw_sb, den8_sb)
                        elif hp == 3:
                            normalize_half(qc, 1, yraw_sb, den8_sb)

                # units in twos: both S pairs back-to-back so the second
                # unit's LDWEIGHTS overlaps the first's member-B stream
                # (disjoint row groups); pairs have an even unit count so
                # groups never straddle an hp boundary
                nfeed = 8 if qc == 0 else 3
                for ua, ub in zip(units[0::2], units[1::2]):
                    for u in (ua, ub):
                        require(("qk", qc, u[0]))
                        require(("qk", u[1] // 4, 4 + u[0]))
                    sa = emit_S(qc, ua)
                    sb = emit_S(qc, ub)
                    ea = emit_expmask(qc, ua, sa)
                    eb = emit_expmask(qc, ub, sb)
                    if pend is not None:
                        flush_pair(pend)
                    feed(qc, nfeed)
                    pend = ((ua, ea), (ub, eb))
                flush_pair(pend)
                # stagger outproj due-tags: most groups feed the next chunks'
                # attention; qc3's groups stay for the final drain
                otags = {0: (1, 1, 2, 2), 1: (2, 2, 3, 3),
                         2: (3, 3, 3, 3), 3: (4, 4, 4, 4)}[qc]
                for tt in range(4):
                    key = ("op", qc, tt)
                    gens[key] = outproj_group_gen(qc, tt)
                    soft_q.append((otags[tt], key))
                if qc + 2 < NQC:
                    add_proj_chunk(qc + 2)
            drain_soft()
    nc.compile()
    return nc


def _get_nc():
    if "nc" not in _CACHE:
        _CACHE["nc"] = _build()
    return _CACHE["nc"]


def _host_prep(x, W_attn, W_proj):
    """Shard + lay out per-core inputs. Returns list of 8 in_maps."""
    x = np.asarray(x, dtype=np.float32)
    W_attn = np.asarray(W_attn, dtype=np.float32)
    W_proj = np.asarray(W_proj, dtype=np.float32)

    # triangular mask prefix: mask[s, i] = 1.0 if s <= i else 0
    s_idx = np.arange(128)[:, None]
    q_idx = np.arange(512)[None, :]
    tri = (s_idx <= q_idx).astype(np.float32)
    mask = np.ascontiguousarray(np.concatenate(
        [tri[:, :512], tri[:, :384], tri[:, :256], tri[:, :128]], axis=1
    )).astype(Bb16)

    xT_b = [np.ascontiguousarray(x[b].T).astype(Bb16) for b in range(B)]
    in_maps = []
    for core in range(8):
        b, g = core // 2, core % 2
        c0 = g * 512
        wqk_g = np.concatenate(
            [W_attn[:, c0:c0 + 512], W_attn[:, C + c0:C + c0 + 512]], axis=1
        ).astype(Bb16)
        wv_g = np.ascontiguousarray(
            W_attn[:, 2 * C + c0:2 * C + c0 + 512]).astype(Bb16)
        wp_g = np.ascontiguousarray(W_proj[c0:c0 + 512, :]).astype(Bb16)
        in_maps.append({
            "xT": xT_b[b],
            "wqk": np.ascontiguousarray(wqk_g),
            "wv": wv_g,
            "wp": wp_g,
            "mask": mask,
        })
    return in_maps


def kernel(x, W_attn, W_proj):
    from concourse import bass_utils

    nc = _get_nc()
    in_maps = _host_prep(x, W_attn, W_proj)
    res = bass_utils.run_bass_kernel_spmd(nc, in_maps, core_ids=list(range(8)))
    outs = [res.results[c]["out"] for c in range(8)]
    full = np.empty((B, T, C), dtype=np.float32)
    for b in range(B):
        full[b] = outs[2 * b].astype(np.float32) + outs[2 * b + 1].astype(np.float32)
    return full


# revision 32
# speedup vs baseline: 1.2019x; 1.0306x over previous
"""Causal multi-head self-attention (B=4, T=2048, C=1024, H=16) on 8 TRN2 NeuronCores.

Sharding: core = b*2 + g  (b = batch 0..3, g = head-group 0..1 of 8 heads each).
Data parallel over batch; tensor parallel over heads (column-parallel W_attn,
row-parallel W_proj). Each core returns a partial (T, C) output; the host sums
the two partials per batch (the TP all-reduce happens in the unshard step).

Per-core device kernel (bf16 matmuls, f32 accumulation), per 512-wide q chunk:
  1. qT/kT projection with heads on partitions; head pairs share a 128-row tile
  2. v projection in natural [t, c] layout, strided-copied into a 65-stride
     augmented layout with an all-ones column per head (the ones column turns
     the softmax denominator into row 64 of the y^T psum)
  3. attention in transposed [s, q] layout -- no transposes anywhere:
       S^T block = (kT block)^T @ qT chunk  (head-even rows 0:64 / head-odd rows
       64:128 of the PE array run concurrently: disjoint row groups)
       E = exp(S/8) on ScalarE (no max-subtraction: |scores|/8 < ~7)
       causal mask = precomputed multiplicative 0/1 tile on diagonal blocks
       y^T_aug accumulates v_aug^T @ E over s blocks in PSUM.  Head A's AV is
       row-split into two concurrent K=64 matmuls (same 64x128 tiling mode as
       the S matmuls) accumulating into two banks summed at pair_end; head B
       keeps the full-K form (PSUM budget: 1 mm + 4 s + 3 av = 8 banks).
  4. row-parallel output projection of the finished q chunk.

v3 scheduling: a PE warmup burst of dummy matmuls runs during the initial DMA
wait so the HAM clock gate reaches 8/8 before the first real matmul.  The boot
projection covers only {m0,m4,m1,m5,m2} k-outer plus v block 0; everything
else drains as fillers inside the ACT-paced attention stretches at a higher
feed rate than v2, with outproj due-tags retuned so the qc3 tail stays dense.
"""

import numpy as np
import ml_dtypes

B, T, C, H = 4, 2048, 1024, 16
HS = C // H          # 64
NHL = 8              # local heads per core
KT = C // 128        # 8 contraction subtiles
NQC = T // 512       # 4 query chunks
NTB = T // 128       # 16 t-blocks
Bb16 = ml_dtypes.bfloat16

_CACHE = {}


def _build():
    import concourse.bass as bass
    import concourse.bacc as bacc
    import concourse.tile as tile
    import concourse.mybir as mybir
    from collections import deque

    BF = mybir.dt.bfloat16
    F32 = mybir.dt.float32
    AF = mybir.ActivationFunctionType

    nc = bacc.Bacc("TRN2", target_bir_lowering=False, debug=False, num_devices=8)
    xT = nc.dram_tensor("xT", [C, T], BF, kind="ExternalInput").ap()
    wqk = nc.dram_tensor("wqk", [C, 1024], BF, kind="ExternalInput").ap()
    wv = nc.dram_tensor("wv", [C, 512], BF, kind="ExternalInput").ap()
    wp = nc.dram_tensor("wp", [512, C], BF, kind="ExternalInput").ap()
    mask = nc.dram_tensor("mask", [128, 1280], BF, kind="ExternalInput").ap()
    out = nc.dram_tensor("out", [T, C], BF, kind="ExternalOutput").ap()

    MOFF = [0, 512, 896, 1152]   # mask_sb offsets for diag blocks d=0..3

    with tile.TileContext(nc) as tc:
        with tc.tile_pool(name="persist", bufs=1) as persist, \
             tc.tile_pool(name="mm", bufs=2, space="PSUM") as mmpool, \
             tc.tile_pool(name="s", bufs=2, space="PSUM") as spool, \
             tc.tile_pool(name="av", bufs=2, space="PSUM") as avpool, \
             tc.tile_pool(name="e", bufs=6) as epool, \
             tc.tile_pool(name="nrm", bufs=3) as nrmpool, \
             tc.tile_pool(name="osb", bufs=3) as outpool:

            xT_sb = persist.tile([128, KT, T], BF, tag="xT")
            wqk_sb = persist.tile([128, KT, 1024], BF, tag="wqk")
            wv_sb = persist.tile([128, KT, 512], BF, tag="wv")
            wp_sb = persist.tile([128, 4, 1024], BF, tag="wp")
            mask_sb = persist.tile([128, 1280], BF, tag="mask")
            qk_sb = persist.tile([128, 8, T], BF, tag="qk")
            v_sb = persist.tile([128, NTB, 520], BF, tag="v")
            yT_sb = persist.tile([128, 4, T], BF, tag="yT")
            scr_sb = persist.tile([128, 64], BF, tag="scratch")
            # pair-broadcast weights: w[0,0:64]=w[64,0:64]=1 selects rA
            # into psum partitions 0:64, w[32,64:128]=w[96,64:128]=1 selects rB
            # into 64:128; K=33 slices at base 0 / base 64 match r4's rows
            ones33_sb = persist.tile([97, 128], F32, tag="ones33")
            nc.vector.memset(scr_sb[:], 0.0)
            nc.vector.memset(ones33_sb[:], 0.0)
            for r in (0, 64):
                nc.vector.memset(ones33_sb[r:r + 1, 0:64], 1.0)
                nc.vector.memset(ones33_sb[r + 32:r + 33, 64:128], 1.0)

            # HAM warmup: ~72 dummy matmuls (~4.3us cold) with no DMA deps so
            # the PE clock gate flips to 8/8 while the first inputs stream in
            wm_ps = mmpool.tile([64, 64], F32, tag="mm", name="warmup")
            for _ in range(72):
                nc.tensor.matmul(wm_ps[:], scr_sb[:], scr_sb[:],
                                 start=True, stop=True)

            # load order: exactly what the first projection chunk needs, first
            for k in range(KT):
                nc.sync.dma_start(wqk_sb[:, k, :], wqk[k * 128:(k + 1) * 128, :])
                nc.sync.dma_start(xT_sb[:, k, 0:512], xT[k * 128:(k + 1) * 128, 0:512])
            for k in range(KT):
                nc.sync.dma_start(wv_sb[:, k, :], wv[k * 128:(k + 1) * 128, :])
            nc.sync.dma_start(mask_sb[:], mask[:])
            for k in range(KT):
                nc.sync.dma_start(xT_sb[:, k, 512:1024],
                                  xT[k * 128:(k + 1) * 128, 512:1024])
            for k in range(4):
                nc.sync.dma_start(wp_sb[:, k, :], wp[k * 128:(k + 1) * 128, :])
            for k in range(KT):
                nc.sync.dma_start(xT_sb[:, k, 1024:2048],
                                  xT[k * 128:(k + 1) * 128, 1024:2048])

            # ---- filler generators: ~one PE matmul per step ----
            def qk_group_gen(qc, m):
                q0 = qc * 512
                mm_ps = mmpool.tile([128, 512], F32, tag="mm",
                                    name=f"qkg_{qc}_{m}")
                for k in range(KT):
                    nc.tensor.matmul(
                        mm_ps[:], wqk_sb[:, k, m * 128:(m + 1) * 128],
                        xT_sb[:, k, q0:q0 + 512],
                        start=(k == 0), stop=(k == KT - 1))
                    yield
                nc.vector.tensor_copy(qk_sb[:, m, q0:q0 + 512], mm_ps[:])
                yield

            def v_group_gen(j):
                jj = j * 128
                vps = mmpool.tile([128, 512], F32, tag="mm", name=f"vg_{j}")
                for k in range(KT):
                    nc.tensor.matmul(
                        vps[:], xT_sb[:, k, jj:jj + 128],
                        wv_sb[:, k, :],
                        start=(k == 0), stop=(k == KT - 1))
                    yield
                v3 = v_sb[:, j, :].rearrange("p (h e) -> p h e", e=65)
                with nc.allow_low_precision(reason="v bf16"):
                    nc.vector.tensor_copy(
                        v3[:, :, 0:64],
                        vps[:].rearrange("p (h e) -> p h e", e=64))
                yield
                nc.vector.memset(v3[:, :, 64], 1.0)
                yield

            def outproj_group_gen(qc, tt):
                t0 = (qc * 4 + tt) * 128
                osb = outpool.tile([128, 1024], BF, tag="osb", name=f"og_{qc}_{tt}")
                for n in range(2):
                    ops = mmpool.tile([128, 512], F32, tag="mm",
                                      name=f"op_{qc}_{tt}_{n}")
                    for cp in range(4):
                        nc.tensor.matmul(
                            ops[:], yT_sb[:, cp, t0:t0 + 128],
                            wp_sb[:, cp, n * 512:(n + 1) * 512],
                            start=(cp == 0), stop=(cp == 3))
                        yield
                    with nc.allow_low_precision(reason="bf16 partial output"):
                        nc.vector.tensor_copy(osb[:, n * 512:(n + 1) * 512], ops[:])
                    nc.sync.dma_start(out[t0:t0 + 128, n * 512:(n + 1) * 512],
                                      osb[:, n * 512:(n + 1) * 512])
                    yield

            # ---- producer bookkeeping ----
            # gens: one generator per projection group, drained cooperatively
            # by feed() or force-finished by require() right before the first
            # consumer is emitted (the tile framework orders dependencies by
            # emission order, so producers MUST be emitted first).
            gens = {}
            done = set()
            hard_q = deque()         # (due, key): qkv projection groups
            soft_q = deque()         # (due, key): outproj groups
            active = []              # [key] mid-group: finish it before
                                     # starting another (mm pool is 1 buf;
                                     # interleaving two psum-holding groups
                                     # stalls the PE)

            def _step(key):
                try:
                    next(gens[key])
                    return True
                except StopIteration:
                    done.add(key)
                    if active and active[0] == key:
                        active.clear()
                    return False

            def require(key):
                if key in done:
                    return
                if active and active[0] != key:
                    while _step(active[0]):
                        pass
                if not active:
                    active.append(key)
                while _step(key):
                    pass

            def feed(now, n):
                while n > 0:
                    if active:
                        key = active[0]
                    else:
                        while soft_q and soft_q[0][1] in done:
                            soft_q.popleft()
                        while hard_q and hard_q[0][1] in done:
                            hard_q.popleft()
                        if soft_q and soft_q[0][0] <= now:
                            key = soft_q.popleft()[1]
                            active.append(key)
                        elif hard_q and hard_q[0][0] <= now + 1:
                            # hard proj groups may run one chunk early
                            key = hard_q.popleft()[1]
                            active.append(key)
                        else:
                            return
                    if _step(key):
                        n -= 1

            def add_hard(due, key, gen):
                gens[key] = gen
                hard_q.append((due, key))

            def add_proj_chunk(qc):
                # consumer order: q slots m0/m4 and the new v blocks feed
                # hp0 first; later head pairs follow
                add_hard(qc, ("qk", qc, 0), qk_group_gen(qc, 0))
                add_hard(qc, ("qk", qc, 4), qk_group_gen(qc, 4))
                for j in range(4 * qc, 4 * qc + 4):
                    add_hard(qc, ("v", j), v_group_gen(j))
                for m in (1, 5, 2, 6, 3, 7):
                    add_hard(qc, ("qk", qc, m), qk_group_gen(qc, m))

            def drain_soft():
                while active:
                    if not _step(active[0]):
                        break
                while soft_q:
                    due, key = soft_q.popleft()
                    if key in done:
                        continue
                    while _step(key):
                        pass

            # ---- attention units: one kv block, both heads of a pair ----
            def unit_list(qc):
                units = []
                for hp in range(4):
                    blocks = [(j, 0, 512) for j in range(4 * qc)] + \
                             [(4 * qc + d, 128 * d, 512 - 128 * d) for d in range(4)]
                    nb = len(blocks)
                    for bi, (j, qo, w) in enumerate(blocks):
                        units.append((hp, j, qo, w, bi == 0, bi == nb - 1))
                return units

            def emit_S(qc, u):
                hp, j, qo, w, first, last = u
                q0 = qc * 512
                s = spool.tile([128, 1024], F32, tag="s",
                               name=f"s_{qc}_{hp}_{j}")
                # the two heads target disjoint PE row groups, so they stream
                # through the array concurrently
                for pb, off in ((0, 0), (64, 512)):
                    nc.tensor.matmul(
                        s[:, off:off + w],
                        qk_sb[pb:pb + 64, 4 + hp, j * 128:(j + 1) * 128],
                        qk_sb[pb:pb + 64, hp, q0 + qo:q0 + 512],
                        start=True, stop=True,
                        tile_position=(pb, 0))
                return s

            def emit_expmask(qc, u, s):
                hp, j, qo, w, first, last = u
                e = epool.tile([128, 1024], BF, tag="e", name=f"e_{qc}_{hp}_{j}")
                s3 = s.rearrange("p (g q) -> p g q", g=2)[:, :, 0:w]
                e3 = e.rearrange("p (g q) -> p g q", g=2)[:, :, 0:w]
                nc.scalar.activation(e3, s3, AF.Exp, scale=0.125)
                if j >= 4 * qc:
                    moff = MOFF[j - 4 * qc]
                    for off in (0, 512):
                        nc.vector.tensor_mul(
                            e[:, off:off + w], e[:, off:off + w],
                            mask_sb[:, moff:moff + w])
                return e

            def emit_AV(qc, u, e, avA, avB):
                hp, j, qo, w, first, last = u
                for av, off, h in ((avA, 0, 2 * hp), (avB, 512, 2 * hp + 1)):
                    nc.tensor.matmul(
                        av[:, qo:512], v_sb[:, j, h * 65:h * 65 + 65],
                        e[:, off:off + w],
                        start=first, stop=last)

            def pair_end(qc, hp, avA, avB, yraw_sb, den8_sb):
                # stash y and denominator; av psum slots free right away
                for h, av_ps in ((2 * hp, avA), (2 * hp + 1, avB)):
                    with nc.allow_low_precision(reason="attention y bf16"):
                        nc.vector.tensor_copy(yraw_sb[:, h, :], av_ps[0:64, :])
                    p32 = (h % 4) * 32
                    nc.vector.tensor_copy(
                        den8_sb[p32:p32 + 1, h // 4, :], av_ps[64:65, :])

            def normalize_half(qc, half, yraw_sb, den8_sb):
                # heads 4*half .. 4*half+3 finished: reciprocal + scale them.
                # One K=33 fp32 matmul per head-pair broadcasts both heads'
                # reciprocal rows straight out of r4 (rows 1-31 are finite 1.0
                # from the den8 memset, zero-weighted).
                q0 = qc * 512
                r4_sb = nrmpool.tile([128, 512], F32, tag="r4",
                                     name=f"r4_{qc}_{half}")
                nc.vector.reciprocal_approx_fast(r4_sb[:], den8_sb[:, half, :])
                for hp in (2 * half, 2 * half + 1):
                    base = (hp % 2) * 64
                    bc_ps = avpool.tile([128, 512], F32, tag="av",
                                        name=f"bc_{qc}_{hp}")
                    with nc.allow_low_precision(reason="fp32r broadcast"):
                        nc.tensor.matmul(
                            bc_ps[:], ones33_sb[base:base + 33, :],
                            r4_sb[base:base + 33, :], start=True, stop=True)
                    with nc.allow_low_precision(reason="attention y bf16"):
                        nc.vector.tensor_mul(
                            yT_sb[0:64, hp, q0:q0 + 512],
                            yraw_sb[:, 2 * hp, :], bc_ps[0:64, :])
                        nc.vector.tensor_mul(
                            yT_sb[64:128, hp, q0:q0 + 512],
                            yraw_sb[:, 2 * hp + 1, :], bc_ps[64:128, :])

            # ---- main schedule ----
            # boot: chunk-0 qT/kT projection, k as the OUTER loop over 6
            # column slots (2 mm bufs + 4 s-tile halves) so the first matmuls
            # issue after only the first k-slice of DMA; the remaining slots
            # and v blocks 0-3 drain as fillers inside qc0's attention.
            BOOT = [0, 4, 1, 5, 2, 6]
            ps = []
            for idx, m in enumerate(BOOT):
                if idx < 2:
                    ps.append(mmpool.tile([128, 512], F32, tag="mm",
                                          name=f"boot{m}"))
                else:
                    if idx % 2 == 0:
                        st = spool.tile([128, 1024], F32, tag="s",
                                        name=f"boot{m}")
                    ps.append(st[:, (idx % 2) * 512:(idx % 2) * 512 + 512])
            for k in range(KT):
                for idx, m in enumerate(BOOT):
                    nc.tensor.matmul(
                        ps[idx], wqk_sb[:, k, m * 128:(m + 1) * 128],
                        xT_sb[:, k, 0:512],
                        start=(k == 0), stop=(k == KT - 1))
            for idx, m in enumerate(BOOT):
                nc.vector.tensor_copy(qk_sb[:, m, 0:512], ps[idx])
                done.add(("qk", 0, m))
            # remaining qc0 projection work, ordered by first consumer:
            # v0-3 feed hp0's AVs, m3/m7 feed hp3's S
            for j in (0, 1, 2, 3):
                add_hard(0, ("v", j), v_group_gen(j))
            for m in (3, 7):
                add_hard(0, ("qk", 0, m), qk_group_gen(0, m))
            add_proj_chunk(1)

            for qc in range(NQC):
                yraw_sb = nrmpool.tile([64, NHL, 512], BF, tag="yraw",
                                       name=f"yraw{qc}")
                den8_sb = nrmpool.tile([128, 2, 512], F32, tag="den8",
                                       name=f"den8{qc}")
                nc.vector.memset(den8_sb[:], 1.0)
                units = unit_list(qc)
                pend = None      # ((unit, e), (unit, e))
                cur = {}

                def flush_pair(pend):
                    (ua, ea), (ub, eb) = pend
                    require(("v", ua[1]))
                    require(("v", ub[1]))
                    if ua[4]:    # first unit of its head pair
                        cur["avA"] = avpool.tile([65, 512], F32, tag="av",
                                                 name=f"avA_{qc}_{ua[0]}")
                        cur["avB"] = avpool.tile([65, 512], F32, tag="av",
                                                 name=f"avB_{qc}_{ua[0]}")
                    emit_AV(qc, ua, ea, cur["avA"], cur["avB"])
                    emit_AV(qc, ub, eb, cur["avA"], cur["avB"])
                    if ub[5]:    # last unit of its head pair
                        hp = ub[0]
                        pair_end(qc, hp, cur["avA"], cur["avB"],
                                 yraw_sb, den8_sb)
                        if hp == 1:
                            normalize_half(qc, 0, yraw_sb, den8_sb)
                        elif hp == 3:
                            normalize_half(qc, 1, yraw_sb, den8_sb)

                # units in twos: both S pairs back-to-back so the second
                # unit's LDWEIGHTS overlaps the first's member-B stream
                # (disjoint row groups); pairs have an even unit count so
                # groups never straddle an hp boundary
                nfeed = 8 if qc == 0 else 4
                for ua, ub in zip(units[0::2], units[1::2]):
                    for u in (ua, ub):
                        require(("qk", qc, u[0]))
                        require(("qk", u[1] // 4, 4 + u[0]))
                    sa = emit_S(qc, ua)
                    sb = emit_S(qc, ub)
                    ea = emit_expmask(qc, ua, sa)
                    eb = emit_expmask(qc, ub, sb)
                    if pend is not None:
                        flush_pair(pend)
                    feed(qc, nfeed)
                    pend = ((ua, ea), (ub, eb))
                flush_pair(pend)
                # stagger outproj due-tags: most groups feed the next chunks'
                # attention; qc3's groups stay for the final drain
                otags = {0: (1, 1, 2, 2), 1: (2, 2, 3, 3),
                         2: (3, 3, 3, 3), 3: (4, 4, 4, 4)}[qc]
                for tt in range(4):
                    key = ("op", qc, tt)
                    gens[key] = outproj_group_gen(qc, tt)
                    soft_q.append((otags[tt], key))
                if qc + 2 < NQC:
                    add_proj_chunk(qc + 2)
            drain_soft()
    nc.compile()
    return nc


def _get_nc():
    if "nc" not in _CACHE:
        _CACHE["nc"] = _build()
    return _CACHE["nc"]


def _host_prep(x, W_attn, W_proj):
    """Shard + lay out per-core inputs. Returns list of 8 in_maps."""
    x = np.asarray(x, dtype=np.float32)
    W_attn = np.asarray(W_attn, dtype=np.float32)
    W_proj = np.asarray(W_proj, dtype=np.float32)

    # triangular mask prefix: mask[s, i] = 1.0 if s <= i else 0
    s_idx = np.arange(128)[:, None]
    q_idx = np.arange(512)[None, :]
    tri = (s_idx <= q_idx).astype(np.float32)
    mask = np.ascontiguousarray(np.concatenate(
        [tri[:, :512], tri[:, :384], tri[:, :256], tri[:, :128]], axis=1
    )).astype(Bb16)

    xT_b = [np.ascontiguousarray(x[b].T).astype(Bb16) for b in range(B)]
    in_maps = []
    for core in range(8):
        b, g = core // 2, core % 2
        c0 = g * 512
        wqk_g = np.concatenate(
            [W_attn[:, c0:c0 + 512], W_attn[:, C + c0:C + c0 + 512]], axis=1
        ).astype(Bb16)
        wv_g = np.ascontiguousarray(
            W_attn[:, 2 * C + c0:2 * C + c0 + 512]).astype(Bb16)
        wp_g = np.ascontiguousarray(W_proj[c0:c0 + 512, :]).astype(Bb16)
        in_maps.append({
            "xT": xT_b[b],
            "wqk": np.ascontiguousarray(wqk_g),
            "wv": wv_g,
            "wp": wp_g,
            "mask": mask,
        })
    return in_maps


def kernel(x, W_attn, W_proj):
    from concourse import bass_utils

    nc = _get_nc()
    in_maps = _host_prep(x, W_attn, W_proj)
    res = bass_utils.run_bass_kernel_spmd(nc, in_maps, core_ids=list(range(8)))
    outs = [res.results[c]["out"] for c in range(8)]
    full = np.empty((B, T, C), dtype=np.float32)
    for b in range(B):
        full[b] = outs[2 * b].astype(np.float32) + outs[2 * b + 1].astype(np.float32)
    return full


# revision 34
# speedup vs baseline: 1.2676x; 1.0547x over previous
"""Causal multi-head self-attention (B=4, T=2048, C=1024, H=16) on 8 TRN2 NeuronCores.

Sharding: core = b*2 + g  (b = batch 0..3, g = head-group 0..1 of 8 heads each).
Data parallel over batch; tensor parallel over heads (column-parallel W_attn,
row-parallel W_proj). Each core returns a partial (T, C) output; the host sums
the two partials per batch (the TP all-reduce happens in the unshard step).

Per-core device kernel (bf16 matmuls, f32 accumulation), per 512-wide q chunk:
  1. qT/kT projection with heads on partitions; head pairs share a 128-row tile
  2. v projection in natural [t, c] layout, strided-copied into a 65-stride
     augmented layout with an all-ones column per head (the ones column turns
     the softmax denominator into row 64 of the y^T psum)
  3. attention in transposed [s, q] layout -- no transposes anywhere:
       S^T block = (kT block)^T @ qT chunk  (head-even rows 0:64 / head-odd rows
       64:128 of the PE array run concurrently: disjoint row groups)
       E = exp(S/8) on ScalarE (no max-subtraction: |scores|/8 < ~7)
       causal mask = precomputed multiplicative 0/1 tile on diagonal blocks
       y^T_aug accumulates v_aug^T @ E over s blocks in PSUM.  Head A's AV is
       row-split into two concurrent K=64 matmuls (same 64x128 tiling mode as
       the S matmuls) accumulating into two banks summed at pair_end; head B
       keeps the full-K form (PSUM budget: 1 mm + 4 s + 3 av = 8 banks).
  4. row-parallel output projection of the finished q chunk.

v3 scheduling: a PE warmup burst of dummy matmuls runs during the initial DMA
wait so the HAM clock gate reaches 8/8 before the first real matmul.  The boot
projection covers only {m0,m4,m1,m5,m2} k-outer plus v block 0; everything
else drains as fillers inside the ACT-paced attention stretches at a higher
feed rate than v2, with outproj due-tags retuned so the qc3 tail stays dense.
"""

import numpy as np
import ml_dtypes

B, T, C, H = 4, 2048, 1024, 16
HS = C // H          # 64
NHL = 8              # local heads per core
KT = C // 128        # 8 contraction subtiles
NQC = T // 512       # 4 query chunks
NTB = T // 128       # 16 t-blocks
Bb16 = ml_dtypes.bfloat16

_CACHE = {}


def _build():
    import concourse.bass as bass
    import concourse.bacc as bacc
    import concourse.tile as tile
    import concourse.mybir as mybir
    from collections import deque

    BF = mybir.dt.bfloat16
    F32 = mybir.dt.float32
    AF = mybir.ActivationFunctionType

    nc = bacc.Bacc("TRN2", target_bir_lowering=False, debug=False, num_devices=8)
    xT = nc.dram_tensor("xT", [C, T], BF, kind="ExternalInput").ap()
    wqk = nc.dram_tensor("wqk", [C, 1024], BF, kind="ExternalInput").ap()
    wv = nc.dram_tensor("wv", [C, 512], BF, kind="ExternalInput").ap()
    wp = nc.dram_tensor("wp", [512, C], BF, kind="ExternalInput").ap()
    mask = nc.dram_tensor("mask", [128, 1280], BF, kind="ExternalInput").ap()
    out = nc.dram_tensor("out", [T, C], BF, kind="ExternalOutput").ap()

    MOFF = [0, 512, 896, 1152]   # mask_sb offsets for diag blocks d=0..3

    with tile.TileContext(nc) as tc:
        with tc.tile_pool(name="persist", bufs=1) as persist, \
             tc.tile_pool(name="mm", bufs=2, space="PSUM") as mmpool, \
             tc.tile_pool(name="s", bufs=2, space="PSUM") as spool, \
             tc.tile_pool(name="av", bufs=2, space="PSUM") as avpool, \
             tc.tile_pool(name="e", bufs=6) as epool, \
             tc.tile_pool(name="nrm", bufs=3) as nrmpool, \
             tc.tile_pool(name="osb", bufs=3) as outpool:

            xT_sb = persist.tile([128, KT, T], BF, tag="xT")
            wqk_sb = persist.tile([128, KT, 1024], BF, tag="wqk")
            wv_sb = persist.tile([128, KT, 512], BF, tag="wv")
            wp_sb = persist.tile([128, 4, 1024], BF, tag="wp")
            mask_sb = persist.tile([128, 1280], BF, tag="mask")
            qk_sb = persist.tile([128, 8, T], BF, tag="qk")
            v_sb = persist.tile([128, NTB, 520], BF, tag="v")
            yT_sb = persist.tile([128, 4, T], BF, tag="yT")
            scr_sb = persist.tile([128, 64], BF, tag="scratch")
            # pair-broadcast weights: w[0,0:64]=w[64,0:64]=1 selects rA
            # into psum partitions 0:64, w[32,64:128]=w[96,64:128]=1 selects rB
            # into 64:128; K=33 slices at base 0 / base 64 match r4's rows
            ones33_sb = persist.tile([97, 128], F32, tag="ones33")
            nc.vector.memset(scr_sb[:], 0.0)
            nc.vector.memset(ones33_sb[:], 0.0)
            for r in (0, 64):
                nc.vector.memset(ones33_sb[r:r + 1, 0:64], 1.0)
                nc.vector.memset(ones33_sb[r + 32:r + 33, 64:128], 1.0)

            # HAM warmup: ~72 dummy matmuls (~4.3us cold) with no DMA deps so
            # the PE clock gate flips to 8/8 while the first inputs stream in
            wm_ps = mmpool.tile([64, 64], F32, tag="mm", name="warmup")
            for _ in range(72):
                nc.tensor.matmul(wm_ps[:], scr_sb[:], scr_sb[:],
                                 start=True, stop=True)

            # load order: exactly what the first projection chunk needs, first
            for k in range(KT):
                nc.sync.dma_start(wqk_sb[:, k, :], wqk[k * 128:(k + 1) * 128, :])
                nc.sync.dma_start(xT_sb[:, k, 0:512], xT[k * 128:(k + 1) * 128, 0:512])
            for k in range(KT):
                nc.sync.dma_start(wv_sb[:, k, :], wv[k * 128:(k + 1) * 128, :])
            nc.sync.dma_start(mask_sb[:], mask[:])
            for k in range(KT):
                nc.sync.dma_start(xT_sb[:, k, 512:1024],
                                  xT[k * 128:(k + 1) * 128, 512:1024])
            for k in range(4):
                nc.sync.dma_start(wp_sb[:, k, :], wp[k * 128:(k + 1) * 128, :])
            for k in range(KT):
                nc.sync.dma_start(xT_sb[:, k, 1024:2048],
                                  xT[k * 128:(k + 1) * 128, 1024:2048])

            # ---- filler generators: ~one PE matmul per step ----
            def qk_group_gen(qc, m):
                q0 = qc * 512
                mm_ps = mmpool.tile([128, 512], F32, tag="mm",
                                    name=f"qkg_{qc}_{m}")
                for k in range(KT):
                    nc.tensor.matmul(
                        mm_ps[:], wqk_sb[:, k, m * 128:(m + 1) * 128],
                        xT_sb[:, k, q0:q0 + 512],
                        start=(k == 0), stop=(k == KT - 1))
                    yield
                nc.vector.tensor_copy(qk_sb[:, m, q0:q0 + 512], mm_ps[:])
                yield

            def v_group_gen(j):
                jj = j * 128
                vps = mmpool.tile([128, 512], F32, tag="mm", name=f"vg_{j}")
                for k in range(KT):
                    nc.tensor.matmul(
                        vps[:], xT_sb[:, k, jj:jj + 128],
                        wv_sb[:, k, :],
                        start=(k == 0), stop=(k == KT - 1))
                    yield
                v3 = v_sb[:, j, :].rearrange("p (h e) -> p h e", e=65)
                with nc.allow_low_precision(reason="v bf16"):
                    nc.vector.tensor_copy(
                        v3[:, :, 0:64],
                        vps[:].rearrange("p (h e) -> p h e", e=64))
                yield
                nc.vector.memset(v3[:, :, 64], 1.0)
                yield

            def outproj_group_gen(qc, tt):
                t0 = (qc * 4 + tt) * 128
                osb = outpool.tile([128, 1024], BF, tag="osb", name=f"og_{qc}_{tt}")
                for n in range(2):
                    ops = mmpool.tile([128, 512], F32, tag="mm",
                                      name=f"op_{qc}_{tt}_{n}")
                    for cp in range(4):
                        nc.tensor.matmul(
                            ops[:], yT_sb[:, cp, t0:t0 + 128],
                            wp_sb[:, cp, n * 512:(n + 1) * 512],
                            start=(cp == 0), stop=(cp == 3))
                        yield
                    with nc.allow_low_precision(reason="bf16 partial output"):
                        nc.vector.tensor_copy(osb[:, n * 512:(n + 1) * 512], ops[:])
                    nc.sync.dma_start(out[t0:t0 + 128, n * 512:(n + 1) * 512],
                                      osb[:, n * 512:(n + 1) * 512])
                    yield

            # ---- producer bookkeeping ----
            # gens: one generator per projection group, drained cooperatively
            # by feed() or force-finished by require() right before the first
            # consumer is emitted (the tile framework orders dependencies by
            # emission order, so producers MUST be emitted first).
            gens = {}
            done = set()
            hard_q = deque()         # (due, key): qkv projection groups
            soft_q = deque()         # (due, key): outproj groups
            active = []              # [key] mid-group: finish it before
                                     # starting another (mm pool is 1 buf;
                                     # interleaving two psum-holding groups
                                     # stalls the PE)

            def _step(key):
                try:
                    next(gens[key])
                    return True
                except StopIteration:
                    done.add(key)
                    if active and active[0] == key:
                        active.clear()
                    return False

            def require(key):
                if key in done:
                    return
                if active and active[0] != key:
                    while _step(active[0]):
                        pass
                if not active:
                    active.append(key)
                while _step(key):
                    pass

            def feed(now, n):
                # hard-first: projection groups have consumers with real
                # deadlines (require() would otherwise burst them right
                # before the consumer, stalling the exp pipeline); outproj
                # is elastic until the final drain
                while n > 0:
                    if active:
                        key = active[0]
                    else:
                        while soft_q and soft_q[0][1] in done:
                            soft_q.popleft()
                        while hard_q and hard_q[0][1] in done:
                            hard_q.popleft()
                        if hard_q and hard_q[0][0] <= now + 1:
                            # hard proj groups may run one chunk early
                            key = hard_q.popleft()[1]
                            active.append(key)
                        elif soft_q and soft_q[0][0] <= now:
                            key = soft_q.popleft()[1]
                            active.append(key)
                        else:
                            return
                    if _step(key):
                        n -= 1

            def add_hard(due, key, gen):
                gens[key] = gen
                hard_q.append((due, key))

            def add_proj_chunk(qc):
                # consumer order: q slots m0/m4 and the new v blocks feed
                # hp0 first; later head pairs follow
                add_hard(qc, ("qk", qc, 0), qk_group_gen(qc, 0))
                add_hard(qc, ("qk", qc, 4), qk_group_gen(qc, 4))
                for j in range(4 * qc, 4 * qc + 4):
                    add_hard(qc, ("v", j), v_group_gen(j))
                for m in (1, 5, 2, 6, 3, 7):
                    add_hard(qc, ("qk", qc, m), qk_group_gen(qc, m))

            def drain_soft():
                while active:
                    if not _step(active[0]):
                        break
                while soft_q:
                    due, key = soft_q.popleft()
                    if key in done:
                        continue
                    while _step(key):
                        pass

            # ---- attention units: one kv block, both heads of a pair ----
            def unit_list(qc):
                units = []
                for hp in range(4):
                    blocks = [(j, 0, 512) for j in range(4 * qc)] + \
                             [(4 * qc + d, 128 * d, 512 - 128 * d) for d in range(4)]
                    nb = len(blocks)
                    for bi, (j, qo, w) in enumerate(blocks):
                        units.append((hp, j, qo, w, bi == 0, bi == nb - 1))
                return units

            def emit_S(qc, u):
                hp, j, qo, w, first, last = u
                q0 = qc * 512
                s = spool.tile([128, 1024], F32, tag="s",
                               name=f"s_{qc}_{hp}_{j}")
                # the two heads target disjoint PE row groups, so they stream
                # through the array concurrently
                for pb, off in ((0, 0), (64, 512)):
                    nc.tensor.matmul(
                        s[:, off:off + w],
                        qk_sb[pb:pb + 64, 4 + hp, j * 128:(j + 1) * 128],
                        qk_sb[pb:pb + 64, hp, q0 + qo:q0 + 512],
                        start=True, stop=True,
                        tile_position=(pb, 0))
                return s

            def emit_expmask(qc, u, s):
                hp, j, qo, w, first, last = u
                e = epool.tile([128, 1024], BF, tag="e", name=f"e_{qc}_{hp}_{j}")
                s3 = s.rearrange("p (g q) -> p g q", g=2)[:, :, 0:w]
                e3 = e.rearrange("p (g q) -> p g q", g=2)[:, :, 0:w]
                nc.scalar.activation(e3, s3, AF.Exp, scale=0.125)
                if j >= 4 * qc:
                    moff = MOFF[j - 4 * qc]
                    for off in (0, 512):
                        nc.vector.tensor_mul(
                            e[:, off:off + w], e[:, off:off + w],
                            mask_sb[:, moff:moff + w])
                return e

            def emit_AV(qc, u, e, avA, avB):
                hp, j, qo, w, first, last = u
                for av, off, h in ((avA, 0, 2 * hp), (avB, 512, 2 * hp + 1)):
                    nc.tensor.matmul(
                        av[:, qo:512], v_sb[:, j, h * 65:h * 65 + 65],
                        e[:, off:off + w],
                        start=first, stop=last)

            def pair_end(qc, hp, avA, avB, yraw_sb, den8_sb):
                # stash y and denominator; av psum slots free right away
                for h, av_ps in ((2 * hp, avA), (2 * hp + 1, avB)):
                    with nc.allow_low_precision(reason="attention y bf16"):
                        nc.vector.tensor_copy(yraw_sb[:, h, :], av_ps[0:64, :])
                    p32 = (h % 4) * 32
                    nc.vector.tensor_copy(
                        den8_sb[p32:p32 + 1, h // 4, :], av_ps[64:65, :])

            def normalize_half(qc, half, yraw_sb, den8_sb):
                # heads 4*half .. 4*half+3 finished: reciprocal + scale them.
                # One K=33 fp32 matmul per head-pair broadcasts both heads'
                # reciprocal rows straight out of r4 (rows 1-31 are finite 1.0
                # from the den8 memset, zero-weighted).
                q0 = qc * 512
                r4_sb = nrmpool.tile([128, 512], F32, tag="r4",
                                     name=f"r4_{qc}_{half}")
                nc.vector.reciprocal_approx_fast(r4_sb[:], den8_sb[:, half, :])
                for hp in (2 * half, 2 * half + 1):
                    base = (hp % 2) * 64
                    bc_ps = avpool.tile([128, 512], F32, tag="av",
                                        name=f"bc_{qc}_{hp}")
                    with nc.allow_low_precision(reason="fp32r broadcast"):
                        nc.tensor.matmul(
                            bc_ps[:], ones33_sb[base:base + 33, :],
                            r4_sb[base:base + 33, :], start=True, stop=True)
                    with nc.allow_low_precision(reason="attention y bf16"):
                        nc.vector.tensor_mul(
                            yT_sb[0:64, hp, q0:q0 + 512],
                            yraw_sb[:, 2 * hp, :], bc_ps[0:64, :])
                        nc.vector.tensor_mul(
                            yT_sb[64:128, hp, q0:q0 + 512],
                            yraw_sb[:, 2 * hp + 1, :], bc_ps[64:128, :])

            # ---- main schedule ----
            # boot: chunk-0 qT/kT projection, k as the OUTER loop over 6
            # column slots (2 mm bufs + 4 s-tile halves) so the first matmuls
            # issue after only the first k-slice of DMA; the remaining slots
            # and v blocks 0-3 drain as fillers inside qc0's attention.
            BOOT = [0, 4, 1, 5, 2, 6]
            ps = []
            for idx, m in enumerate(BOOT):
                if idx < 2:
                    ps.append(mmpool.tile([128, 512], F32, tag="mm",
                                          name=f"boot{m}"))
                else:
                    if idx % 2 == 0:
                        st = spool.tile([128, 1024], F32, tag="s",
                                        name=f"boot{m}")
                    ps.append(st[:, (idx % 2) * 512:(idx % 2) * 512 + 512])
            for k in range(KT):
                for idx, m in enumerate(BOOT):
                    nc.tensor.matmul(
                        ps[idx], wqk_sb[:, k, m * 128:(m + 1) * 128],
                        xT_sb[:, k, 0:512],
                        start=(k == 0), stop=(k == KT - 1))
            for idx, m in enumerate(BOOT):
                nc.vector.tensor_copy(qk_sb[:, m, 0:512], ps[idx])
                done.add(("qk", 0, m))
            # remaining qc0 projection work, ordered by first consumer:
            # v0-3 feed hp0's AVs, m3/m7 feed hp3's S
            for j in (0, 1, 2, 3):
                add_hard(0, ("v", j), v_group_gen(j))
            for m in (3, 7):
                add_hard(0, ("qk", 0, m), qk_group_gen(0, m))
            add_proj_chunk(1)

            for qc in range(NQC):
                yraw_sb = nrmpool.tile([64, NHL, 512], BF, tag="yraw",
                                       name=f"yraw{qc}")
                den8_sb = nrmpool.tile([128, 2, 512], F32, tag="den8",
                                       name=f"den8{qc}")
                nc.vector.memset(den8_sb[:], 1.0)
                units = unit_list(qc)
                pend = None      # ((unit, e), (unit, e))
                cur = {}

                def flush_pair(pend):
                    (ua, ea), (ub, eb) = pend
                    require(("v", ua[1]))
                    require(("v", ub[1]))
                    if ua[4]:    # first unit of its head pair
                        cur["avA"] = avpool.tile([65, 512], F32, tag="av",
                                                 name=f"avA_{qc}_{ua[0]}")
                        cur["avB"] = avpool.tile([65, 512], F32, tag="av",
                                                 name=f"avB_{qc}_{ua[0]}")
                    emit_AV(qc, ua, ea, cur["avA"], cur["avB"])
                    emit_AV(qc, ub, eb, cur["avA"], cur["avB"])
                    if ub[5]:    # last unit of its head pair
                        hp = ub[0]
                        pair_end(qc, hp, cur["avA"], cur["avB"],
                                 yraw_sb, den8_sb)
                        if hp == 1:
                            normalize_half(qc, 0, yraw_sb, den8_sb)
                        elif hp == 3:
                            normalize_half(qc, 1, yraw_sb, den8_sb)

                # units in twos: both S pairs back-to-back so the second
                # unit's LDWEIGHTS overlaps the first's member-B stream
                # (disjoint row groups); pairs have an even unit count so
                # groups never straddle an hp boundary
                nfeed = 8 if qc == 0 else 4
                for ua, ub in zip(units[0::2], units[1::2]):
                    for u in (ua, ub):
                        require(("qk", qc, u[0]))
                        require(("qk", u[1] // 4, 4 + u[0]))
                    sa = emit_S(qc, ua)
                    sb = emit_S(qc, ub)
                    ea = emit_expmask(qc, ua, sa)
                    eb = emit_expmask(qc, ub, sb)
                    if pend is not None:
                        flush_pair(pend)
                    feed(qc, nfeed)
                    pend = ((ua, ea), (ub, eb))
                flush_pair(pend)
                # stagger outproj due-tags: most groups feed the next chunks'
                # attention; qc3's groups stay for the final drain
                # qc2's last two groups stay for the final drain so the PE
                # has work across the last normalize chain
                otags = {0: (1, 1, 2, 2), 1: (2, 2, 3, 3),
                         2: (3, 3, 4, 4), 3: (4, 4, 4, 4)}[qc]
                for tt in range(4):
                    key = ("op", qc, tt)
                    gens[key] = outproj_group_gen(qc, tt)
                    soft_q.append((otags[tt], key))
                if qc + 2 < NQC:
                    add_proj_chunk(qc + 2)
            drain_soft()
    nc.compile()
    return nc


def _get_nc():
    if "nc" not in _CACHE:
        _CACHE["nc"] = _build()
    return _CACHE["nc"]


def _host_prep(x, W_attn, W_proj):
    """Shard + lay out per-core inputs. Returns list of 8 in_maps."""
    x = np.asarray(x, dtype=np.float32)
    W_attn = np.asarray(W_attn, dtype=np.float32)
    W_proj = np.asarray(W_proj, dtype=np.float32)

    # triangular mask prefix: mask[s, i] = 1.0 if s <= i else 0
    s_idx = np.arange(128)[:, None]
    q_idx = np.arange(512)[None, :]
    tri = (s_idx <= q_idx).astype(np.float32)
    mask = np.ascontiguousarray(np.concatenate(
        [tri[:, :512], tri[:, :384], tri[:, :256], tri[:, :128]], axis=1
    )).astype(Bb16)

    xT_b = [np.ascontiguousarray(x[b].T).astype(Bb16) for b in range(B)]
    in_maps = []
    for core in range(8):
        b, g = core // 2, core % 2
        c0 = g * 512
        wqk_g = np.concatenate(
            [W_attn[:, c0:c0 + 512], W_attn[:, C + c0:C + c0 + 512]], axis=1
        ).astype(Bb16)
        wv_g = np.ascontiguousarray(
            W_attn[:, 2 * C + c0:2 * C + c0 + 512]).astype(Bb16)
        wp_g = np.ascontiguousarray(W_proj[c0:c0 + 512, :]).astype(Bb16)
        in_maps.append({
            "xT": xT_b[b],
            "wqk": np.ascontiguousarray(wqk_g),
            "wv": wv_g,
            "wp": wp_g,
            "mask": mask,
        })
    return in_maps


def kernel(x, W_attn, W_proj):
    from concourse import bass_utils

    nc = _get_nc()
    in_maps = _host_prep(x, W_attn, W_proj)
    res = bass_utils.run_bass_kernel_spmd(nc, in_maps, core_ids=list(range(8)))
    outs = [res.results[c]["out"] for c in range(8)]
    full = np.empty((B, T, C), dtype=np.float32)
    for b in range(B):
        full[b] = outs[2 * b].astype(np.float32) + outs[2 * b + 1].astype(np.float32)
    return full
